# revision 11
# baseline (speedup 1.0000x reference)
"""MultiHeadAttention kernel for 8x Trainium2 NeuronCores.

Strategy: data-parallel over batch (B=8 -> 1 batch element per core, no
collectives). Per core the whole attention block runs in a transposed-layout
dataflow so no large on-chip transposes are needed:

  host:     pass qT,kT,vT ([D,S] per batch) and transposed weights WqT (pre-
            scaled by 1/sqrt(dk)), WkT, WvT, WoT.
  phase 1:  qhT/khT = W.T-stationary matmuls -> [head_dim, S] layouts.
            vhT likewise, then 128x128 PE transposes -> vh [S, head_dim]
            (+ ones column appended for free row-sums later).
  phase 2:  per head:
            S1: scoresT[sj,si] = khT.T @ qhT -> exp -> A_t (unnormalized)
            S2: outhT[d,si] (+ sums row via the ones column) = vh_aug.T @ A_t
                normalize with broadcast reciprocal -> outhT
            S3: scores[si,sj] = qhT.T @ khT -> exp (accum_out = row sums)
                -> normalize -> DMA out the attention output
  phase 3:  out[s,m] = outhT.T @ WoT + bo -> DMA.

Matmuls run as float32r (TF32-like full-rate mode on the PE array).
"""

import numpy as np

B = 8
S = 1024
D = 1024
H = 16
DK = 64
P = 128
NF = 512
N_CORES = 8

KT = D // P  # contraction tiles per projection
OT = D // P  # output row tiles (head pairs)
ST = S // P  # sequence tiles of 128
SH = S // NF  # sequence halves of 512


def _install_drain_patch():
    """This walrus build rejects >1 sync-wait on CTRL (Drain) instructions.
    Split the Tile tail-drain's waits onto single-wait SP nops."""
    import concourse.tile as tile
    from concourse.vector_clock import ScopedClock, VectorClock
    import concourse.tile_sem_assignment as tsa

    if getattr(tile.TileContext, "_drain_patch_installed", False):
        return

    def _patched(self, tick_clock, wait_clock):
        nc = self.nc
        g = tick_clock.global_clock
        N = tsa.N_PROCS
        for p in range(N):
            if g[p]:
                partial = VectorClock([g[q] if q == p else 0 for q in range(N)])
                n = nc.sync.nop(nofuse=True)
                wait_clock.add_sem_waits(n.ins, ScopedClock({None: partial}))
        nc.sync.drain()
        nc.all_engine_barrier()
        assert self.sems is not None
        popped = nc._tile_sem_poison_stack.pop()
        assert popped is self._sem_poison
        nc.clear_and_free_semaphores(list(self.sems.allocated().values()))
        nc.all_engine_barrier()

    tile.TileContext._drain_and_barrier = _patched
    tile.TileContext._drain_patch_installed = True


def _split_multi_waits(nc, limit=1):
    """This walrus build supports at most `limit` sync-waits per instruction.
    Hoist extra waits onto single-wait NoOps inserted just before, on the
    same engine (the sequencer blocks on them, preserving semantics)."""
    from concourse import mybir

    ctr = 0
    for f in nc.m.functions:
        for bb in f.blocks:
            lst = bb.instructions
            i = 0
            while i < len(lst):
                inst = lst[i]
                si = inst.sync_info
                if si is not None and len(si.on_wait) > limit:
                    waits = list(si.on_wait)
                    SI = type(si)
                    keep = waits[-limit:]
                    extra = waits[:-limit]
                    for j in range(0, len(extra), limit):
                        chunk = extra[j:j + limit]
                        ctr += 1
                        nop = mybir.InstNoOp(name=f"WSPLIT-{ctr}", ins=[], outs=[])
                        nop.engine = inst.engine
                        nop.sync_info = SI(on_wait=chunk, on_update=[])
                        lst.insert(i, nop)
                        i += 1
                    inst.sync_info = SI(on_wait=keep, on_update=list(si.on_update))
                i += 1
    return ctr


def build_program(mm_dtype_name="float32r", split_waits=True):
    import concourse.bass as bass
    import concourse.tile as tile
    from concourse import mybir
    from concourse.masks import make_identity
    from contextlib import ExitStack

    _install_drain_patch()

    f32 = mybir.dt.float32
    mdt = getattr(mybir.dt, mm_dtype_name)
    AF = mybir.ActivationFunctionType

    nc = bass.Bass()

    qT = nc.declare_dram_parameter("qT", [D, S], mdt, isOutput=False)
    kT = nc.declare_dram_parameter("kT", [D, S], mdt, isOutput=False)
    vT = nc.declare_dram_parameter("vT", [D, S], mdt, isOutput=False)
    wqT = nc.declare_dram_parameter("wqT", [D, D], mdt, isOutput=False)
    wkT = nc.declare_dram_parameter("wkT", [D, D], mdt, isOutput=False)
    wvT = nc.declare_dram_parameter("wvT", [D, D], mdt, isOutput=False)
    woT = nc.declare_dram_parameter("woT", [D, D], mdt, isOutput=False)
    bq = nc.declare_dram_parameter("bq", [D], f32, isOutput=False)
    bk = nc.declare_dram_parameter("bk", [D], f32, isOutput=False)
    bv = nc.declare_dram_parameter("bv", [D], f32, isOutput=False)
    bo = nc.declare_dram_parameter("bo", [D], f32, isOutput=False)
    onesd = nc.declare_dram_parameter("onesd", [P, ST], mdt, isOutput=False)
    att = nc.declare_dram_parameter("att", [H, S, S], f32, isOutput=True)
    out = nc.declare_dram_parameter("out", [S, D], f32, isOutput=True)

    def col(dram_vec, t):
        return dram_vec[t * P:(t + 1) * P].rearrange("(p o) -> p o", o=1)

    with tile.TileContext(nc) as tc, ExitStack() as ctx:
        const = ctx.enter_context(tc.tile_pool(name="const", bufs=1))
        ident = const.tile([P, P], f32)
        make_identity(nc, ident)

        bcol = {}
        for nm, bh in (("q", bq), ("k", bk), ("v", bv)):
            t = const.tile([P, OT], f32, name="bcol")
            for i in range(OT):
                nc.sync.dma_start(out=t[:, i:i + 1], in_=col(bh, i))
            bcol[nm] = t
        bo_bc = const.tile([P, D], f32)
        nc.gpsimd.dma_start(
            out=bo_bc,
            in_=bo[:].rearrange("(o d) -> o d", o=1).to_broadcast((P, D)),
        )

        # persistent per-head activations
        qhT_p = ctx.enter_context(tc.tile_pool(name="qhT", bufs=1))
        khT_p = ctx.enter_context(tc.tile_pool(name="khT", bufs=1))
        vha_p = ctx.enter_context(tc.tile_pool(name="vha", bufs=1))
        qhT = [qhT_p.tile([P, S], mdt, name=f"qhT{i}") for i in range(OT)]
        khT = [khT_p.tile([P, S], mdt, name=f"khT{i}") for i in range(OT)]
        # vh augmented with a ones column: [sj_partition, sj_tile, dk+1]
        vha = [vha_p.tile([P, ST, DK + 1], mdt, name=f"vha{h}") for h in range(H)]
        for h in range(H):
            nc.sync.dma_start(
                out=vha[h][:, :, DK:DK + 1], in_=onesd[:, :].unsqueeze(2)
            )

        # ---------------- phase 1: projections ----------------
        with tc.tile_pool(name="xin", bufs=KT) as x_p, \
             tc.tile_pool(name="win", bufs=KT) as w_p, \
             tc.tile_pool(name="vhT", bufs=2) as vhT_p, \
             tc.tile_pool(name="ppsum", bufs=4, space="PSUM") as ppsum, \
             tc.tile_pool(name="tpsum", bufs=2, space="PSUM") as tpsum:

            for pi, (xh, wh, bnm) in enumerate(
                ((qT, wqT, "q"), (kT, wkT, "k"), (vT, wvT, "v"))
            ):
                xt = [x_p.tile([P, S], mdt, tag="xt", name="xt") for _ in range(KT)]
                wt = [w_p.tile([P, D], mdt, tag="wt", name="wt") for _ in range(KT)]
                for k_ in range(KT):
                    nc.sync.dma_start(out=xt[k_], in_=xh[k_ * P:(k_ + 1) * P, :])
                    nc.sync.dma_start(out=wt[k_], in_=wh[k_ * P:(k_ + 1) * P, :])
                for o in range(OT):
                    for sh in range(SH):
                        ps = ppsum.tile([P, NF], f32, tag="pp", name="pp")
                        for k_ in range(KT):
                            nc.tensor.matmul(
                                ps,
                                lhsT=wt[k_][:, o * P:(o + 1) * P].bitcast(mdt),
                                rhs=xt[k_][:, sh * NF:(sh + 1) * NF].bitcast(mdt),
                                start=(k_ == 0),
                                stop=(k_ == KT - 1),
                            )
                        if pi < 2:
                            dst = qhT[o] if pi == 0 else khT[o]
                            nc.vector.tensor_scalar_add(
                                out=dst[:, sh * NF:(sh + 1) * NF],
                                in0=ps,
                                scalar1=bcol[bnm][:, o:o + 1],
                            )
                        else:
                            vt = vhT_p.tile([P, NF], f32, tag="vt", name="vt")
                            nc.vector.tensor_scalar_add(
                                out=vt, in0=ps, scalar1=bcol[bnm][:, o:o + 1]
                            )
                            # transpose the head-pair half-block into vh layout
                            for sjl in range(NF // P):
                                sj = sh * (NF // P) + sjl
                                tp = tpsum.tile([P, P], f32, tag="tp", name="tp")
                                nc.tensor.transpose(
                                    tp, vt[:, sjl * P:(sjl + 1) * P], ident
                                )
                                for hp in range(2):
                                    nc.vector.tensor_copy(
                                        out=vha[o * 2 + hp][:, sj, 0:DK],
                                        in_=tp[:, hp * DK:(hp + 1) * DK],
                                    )

        # ---------------- phase 2: attention per head ----------------
        ohT_p = ctx.enter_context(tc.tile_pool(name="ohT", bufs=1))
        ohT = [ohT_p.tile([P, S], mdt, name=f"ohT{i}") for i in range(OT)]
        drs_p = ctx.enter_context(tc.tile_pool(name="drs", bufs=2, space="DRAM"))
        with tc.tile_pool(name="stp", bufs=2, space="PSUM") as stp, \
             tc.tile_pool(name="avp", bufs=4, space="PSUM") as avp, \
             tc.tile_pool(name="snp", bufs=1, space="PSUM") as snp, \
             tc.tile_pool(name="atp", bufs=6) as atp, \
             tc.tile_pool(name="anp", bufs=2) as anp, \
             tc.tile_pool(name="anp2", bufs=2) as anp2, \
             tc.tile_pool(name="rbp", bufs=2) as rbp, \
             tc.tile_pool(name="rrp", bufs=2) as rrp, \
             tc.tile_pool(name="smallp", bufs=8) as smallp:

            for h in range(H):
                pair, off = h // 2, (h % 2) * DK
                av = [avp.tile([DK + 1, NF], f32, tag="av", name="av") for _ in range(2)]
                # S1 + S2: transposed scores -> exp -> AV accumulation
                for sj in range(ST):
                    for sih in range(SH):
                        st = stp.tile([P, NF], f32, tag="st", name="st")
                        nc.tensor.matmul(
                            st,
                            lhsT=khT[pair][off:off + DK, sj * P:(sj + 1) * P].bitcast(mdt),
                            rhs=qhT[pair][off:off + DK, sih * NF:(sih + 1) * NF].bitcast(mdt),
                            start=True,
                            stop=True,
                        )
                        at = atp.tile([P, NF], mdt, tag="at", name="at")
                        nc.scalar.activation(out=at, in_=st, func=AF.Exp)
                        nc.tensor.matmul(
                            av[sih],
                            lhsT=vha[h][:, sj, :].bitcast(mdt),
                            rhs=at.bitcast(mdt),
                            start=(sj == 0),
                            stop=(sj == ST - 1),
                        )
                # normalize outhT with broadcast reciprocal of the sums row
                rrow = rrp.tile([1, S], f32, tag="rrow", name="rrow")
                for sih in range(SH):
                    nc.vector.reciprocal(
                        rrow[:, sih * NF:(sih + 1) * NF], av[sih][DK:DK + 1, :]
                    )
                r_sc = drs_p.tile([1, S], f32, tag="rsc", name="rsc")
                nc.sync.dma_start(out=r_sc, in_=rrow)
                rbc = rbp.tile([DK, S], f32, tag="rbc", name="rbc")
                nc.gpsimd.dma_start(out=rbc, in_=r_sc.to_broadcast((DK, S)))
                for sih in range(SH):
                    nc.vector.tensor_mul(
                        out=ohT[pair][off:off + DK, sih * NF:(sih + 1) * NF],
                        in0=av[sih][0:DK, :],
                        in1=rbc[:, sih * NF:(sih + 1) * NF],
                    )
                # S3: normal-orientation scores -> softmax -> attention output
                for si in range(ST):
                    sn = snp.tile([P, S], f32, tag="sn", name="sn")
                    for sjh in range(SH):
                        nc.tensor.matmul(
                            sn[:, sjh * NF:(sjh + 1) * NF],
                            lhsT=qhT[pair][off:off + DK, si * P:(si + 1) * P].bitcast(mdt),
                            rhs=khT[pair][off:off + DK, sjh * NF:(sjh + 1) * NF].bitcast(mdt),
                            start=True,
                            stop=True,
                        )
                    ane = anp.tile([P, S], f32, tag="ane", name="ane")
                    sums = smallp.tile([P, 1], f32, tag="sums", name="sums")
                    nc.scalar.activation(out=ane, in_=sn, func=AF.Exp, accum_out=sums)
                    rec = smallp.tile([P, 1], f32, tag="rec", name="rec")
                    nc.vector.reciprocal(rec, sums)
                    ann = anp2.tile([P, S], f32, tag="ann", name="ann")
                    nc.gpsimd.tensor_scalar_mul(out=ann, in0=ane, scalar1=rec)
                    nc.sync.dma_start(
                        out=att[h, si * P:(si + 1) * P, :], in_=ann
                    )

        # ---------------- phase 3: output projection ----------------
        with tc.tile_pool(name="wop", bufs=OT) as wo_p, \
             tc.tile_pool(name="fpsum", bufs=4, space="PSUM") as fpsum, \
             tc.tile_pool(name="fop", bufs=3) as fo_p:
            wot = [wo_p.tile([P, D], mdt, tag="wot", name="wot") for _ in range(KT)]
            for k_ in range(KT):
                nc.sync.dma_start(out=wot[k_], in_=woT[k_ * P:(k_ + 1) * P, :])
            for s in range(ST):
                for mh in range(SH):
                    fp = fpsum.tile([P, NF], f32, tag="fp", name="fp")
                    for op in range(OT):
                        nc.tensor.matmul(
                            fp,
                            lhsT=ohT[op][:, s * P:(s + 1) * P].bitcast(mdt),
                            rhs=wot[op][:, mh * NF:(mh + 1) * NF].bitcast(mdt),
                            start=(op == 0),
                            stop=(op == OT - 1),
                        )
                    fo = fo_p.tile([P, NF], f32, tag="fo", name="fo")
                    nc.vector.tensor_add(
                        out=fo, in0=fp, in1=bo_bc[:, mh * NF:(mh + 1) * NF]
                    )
                    nc.sync.dma_start(
                        out=out[s * P:(s + 1) * P, mh * NF:(mh + 1) * NF], in_=fo
                    )

    if split_waits:
        _split_multi_waits(nc)
    return nc


_prog_cache = {}


def get_program(mm_dtype_name="float32r"):
    if mm_dtype_name not in _prog_cache:
        _prog_cache[mm_dtype_name] = build_program(mm_dtype_name)
    return _prog_cache[mm_dtype_name]


def make_in_maps(q, k, v, Wq, bq, Wk, bk, Wv, bv, Wo, bo):
    scale = np.float32(1.0 / np.sqrt(DK))
    f = np.float32
    wqT = np.ascontiguousarray(np.asarray(Wq, f).T * scale)
    wkT = np.ascontiguousarray(np.asarray(Wk, f).T)
    wvT = np.ascontiguousarray(np.asarray(Wv, f).T)
    woT = np.ascontiguousarray(np.asarray(Wo, f).T)
    bq_s = np.ascontiguousarray(np.asarray(bq, f) * scale)
    bk_c = np.ascontiguousarray(np.asarray(bk, f))
    bv_c = np.ascontiguousarray(np.asarray(bv, f))
    bo_c = np.ascontiguousarray(np.asarray(bo, f))
    in_maps = []
    for c in range(N_CORES):
        in_maps.append({
            "qT": np.ascontiguousarray(np.asarray(q[c], f).T),
            "kT": np.ascontiguousarray(np.asarray(k[c], f).T),
            "vT": np.ascontiguousarray(np.asarray(v[c], f).T),
            "wqT": wqT, "wkT": wkT, "wvT": wvT, "woT": woT,
            "bq": bq_s, "bk": bk_c, "bv": bv_c, "bo": bo_c,
            "onesd": np.ones((P, ST), np.float32),
        })
    return in_maps


def kernel(q, k, v, Wq, bq, Wk, bk, Wv, bv, Wo, bo):
    from concourse.bass_utils import run_bass_kernel_spmd

    nc = get_program()
    in_maps = make_in_maps(q, k, v, Wq, bq, Wk, bk, Wv, bv, Wo, bo)
    res = run_bass_kernel_spmd(nc, in_maps, list(range(N_CORES)))
    out = np.stack([res.results[c]["out"] for c in range(N_CORES)])
    attention = np.stack([res.results[c]["att"] for c in range(N_CORES)])
    return out, attention


# revision 12
# speedup vs baseline: 3.0949x; 3.0949x over previous
"""MultiHeadAttention kernel for 8x Trainium2 NeuronCores.

Strategy: data-parallel over batch (B=8 -> 1 batch element per core, no
collectives). Per core the whole attention block runs in a transposed-layout
dataflow so no large on-chip transposes are needed:

  host:     pass qT,kT,vT ([D,S] per batch) and transposed weights WqT (pre-
            scaled by 1/sqrt(dk)), WkT, WvT, WoT.
  phase 1:  qhT/khT = W.T-stationary matmuls -> [head_dim, S] layouts.
            vhT likewise, then 128x128 PE transposes -> vh [S, head_dim]
            (+ ones column appended for free row-sums later).
  phase 2:  per head:
            S1: scoresT[sj,si] = khT.T @ qhT -> exp -> A_t (unnormalized)
            S2: outhT[d,si] (+ sums row via the ones column) = vh_aug.T @ A_t
                normalize with broadcast reciprocal -> outhT
            S3: scores[si,sj] = qhT.T @ khT -> exp (accum_out = row sums)
                -> normalize -> DMA out the attention output
  phase 3:  out[s,m] = outhT.T @ WoT + bo -> DMA.

Matmuls run as float32r (TF32-like full-rate mode on the PE array).
"""

import numpy as np

B = 8
S = 1024
D = 1024
H = 16
DK = 64
P = 128
NF = 512
N_CORES = 8

KT = D // P  # contraction tiles per projection
OT = D // P  # output row tiles (head pairs)
ST = S // P  # sequence tiles of 128
SH = S // NF  # sequence halves of 512


def _install_drain_patch():
    """This walrus build rejects >1 sync-wait on CTRL (Drain) instructions.
    Split the Tile tail-drain's waits onto single-wait SP nops."""
    import concourse.tile as tile
    from concourse.vector_clock import ScopedClock, VectorClock
    import concourse.tile_sem_assignment as tsa

    if getattr(tile.TileContext, "_drain_patch_installed", False):
        return

    def _patched(self, tick_clock, wait_clock):
        nc = self.nc
        g = tick_clock.global_clock
        N = tsa.N_PROCS
        for p in range(N):
            if g[p]:
                partial = VectorClock([g[q] if q == p else 0 for q in range(N)])
                n = nc.sync.nop(nofuse=True)
                wait_clock.add_sem_waits(n.ins, ScopedClock({None: partial}))
        nc.sync.drain()
        nc.all_engine_barrier()
        assert self.sems is not None
        popped = nc._tile_sem_poison_stack.pop()
        assert popped is self._sem_poison
        nc.clear_and_free_semaphores(list(self.sems.allocated().values()))
        nc.all_engine_barrier()

    tile.TileContext._drain_and_barrier = _patched
    tile.TileContext._drain_patch_installed = True


def _split_multi_waits(nc, limit=1):
    """This walrus build supports at most `limit` sync-waits per instruction.
    Hoist extra waits onto single-wait NoOps inserted just before, on the
    same engine (the sequencer blocks on them, preserving semantics)."""
    from concourse import mybir

    ctr = 0
    for f in nc.m.functions:
        for bb in f.blocks:
            lst = bb.instructions
            i = 0
            while i < len(lst):
                inst = lst[i]
                si = inst.sync_info
                if si is not None and len(si.on_wait) > limit:
                    waits = list(si.on_wait)
                    SI = type(si)
                    keep = waits[-limit:]
                    extra = waits[:-limit]
                    for j in range(0, len(extra), limit):
                        chunk = extra[j:j + limit]
                        ctr += 1
                        nop = mybir.InstNoOp(name=f"WSPLIT-{ctr}", ins=[], outs=[])
                        nop.engine = inst.engine
                        nop.sync_info = SI(on_wait=chunk, on_update=[])
                        lst.insert(i, nop)
                        i += 1
                    inst.sync_info = SI(on_wait=keep, on_update=list(si.on_update))
                i += 1
    return ctr


def build_program(mm_dtype_name="float32r", split_waits=True):
    import concourse.bass as bass
    import concourse.tile as tile
    from concourse import mybir
    from concourse.masks import make_identity
    from contextlib import ExitStack

    _install_drain_patch()

    f32 = mybir.dt.float32
    mdt = getattr(mybir.dt, mm_dtype_name)
    AF = mybir.ActivationFunctionType

    nc = bass.Bass()

    qT = nc.declare_dram_parameter("qT", [D, S], mdt, isOutput=False)
    kT = nc.declare_dram_parameter("kT", [D, S], mdt, isOutput=False)
    vT = nc.declare_dram_parameter("vT", [D, S], mdt, isOutput=False)
    wqT = nc.declare_dram_parameter("wqT", [D, D], mdt, isOutput=False)
    wkT = nc.declare_dram_parameter("wkT", [D, D], mdt, isOutput=False)
    wvT = nc.declare_dram_parameter("wvT", [D, D], mdt, isOutput=False)
    woT = nc.declare_dram_parameter("woT", [D, D], mdt, isOutput=False)
    bq = nc.declare_dram_parameter("bq", [D], f32, isOutput=False)
    bk = nc.declare_dram_parameter("bk", [D], f32, isOutput=False)
    bv = nc.declare_dram_parameter("bv", [D], f32, isOutput=False)
    bo = nc.declare_dram_parameter("bo", [D], f32, isOutput=False)
    onesd = nc.declare_dram_parameter("onesd", [P, ST], mdt, isOutput=False)
    att = nc.declare_dram_parameter("att", [H, S, S], f32, isOutput=True)
    out = nc.declare_dram_parameter("out", [S, D], f32, isOutput=True)

    def col(dram_vec, t):
        return dram_vec[t * P:(t + 1) * P].rearrange("(p o) -> p o", o=1)

    with tile.TileContext(nc) as tc, ExitStack() as ctx:
        const = ctx.enter_context(tc.tile_pool(name="const", bufs=1))
        ident = const.tile([P, P], f32)
        make_identity(nc, ident)

        bcol = {}
        for nm, bh in (("q", bq), ("k", bk), ("v", bv)):
            t = const.tile([P, OT], f32, name="bcol")
            for i in range(OT):
                nc.sync.dma_start(out=t[:, i:i + 1], in_=col(bh, i))
            bcol[nm] = t
        bo_bc = const.tile([P, D], f32)
        nc.gpsimd.dma_start(
            out=bo_bc,
            in_=bo[:].rearrange("(o d) -> o d", o=1).to_broadcast((P, D)),
        )

        # persistent per-head activations
        qhT_p = ctx.enter_context(tc.tile_pool(name="qhT", bufs=1))
        khT_p = ctx.enter_context(tc.tile_pool(name="khT", bufs=1))
        vha_p = ctx.enter_context(tc.tile_pool(name="vha", bufs=1))
        qhT = [qhT_p.tile([P, S], mdt, name=f"qhT{i}") for i in range(OT)]
        khT = [khT_p.tile([P, S], mdt, name=f"khT{i}") for i in range(OT)]
        # vh augmented with a ones column: [sj_partition, sj_tile, dk+1]
        vha = [vha_p.tile([P, ST, DK + 1], mdt, name=f"vha{h}") for h in range(H)]
        for h in range(H):
            nc.sync.dma_start(
                out=vha[h][:, :, DK:DK + 1], in_=onesd[:, :].unsqueeze(2)
            )

        # ---------------- phase 1: projections ----------------
        with tc.tile_pool(name="xin", bufs=KT) as x_p, \
             tc.tile_pool(name="win", bufs=KT) as w_p, \
             tc.tile_pool(name="vhT", bufs=2) as vhT_p, \
             tc.tile_pool(name="ppsum", bufs=4, space="PSUM") as ppsum, \
             tc.tile_pool(name="tpsum", bufs=2, space="PSUM") as tpsum:

            for pi, (xh, wh, bnm) in enumerate(
                ((qT, wqT, "q"), (kT, wkT, "k"), (vT, wvT, "v"))
            ):
                xt = [x_p.tile([P, S], mdt, tag="xt", name="xt") for _ in range(KT)]
                wt = [w_p.tile([P, D], mdt, tag="wt", name="wt") for _ in range(KT)]
                for k_ in range(KT):
                    nc.sync.dma_start(out=xt[k_], in_=xh[k_ * P:(k_ + 1) * P, :])
                    nc.sync.dma_start(out=wt[k_], in_=wh[k_ * P:(k_ + 1) * P, :])
                for o in range(OT):
                    for sh in range(SH):
                        ps = ppsum.tile([P, NF], f32, tag="pp", name="pp")
                        for k_ in range(KT):
                            nc.tensor.matmul(
                                ps,
                                lhsT=wt[k_][:, o * P:(o + 1) * P].bitcast(mdt),
                                rhs=xt[k_][:, sh * NF:(sh + 1) * NF].bitcast(mdt),
                                start=(k_ == 0),
                                stop=(k_ == KT - 1),
                            )
                        if pi < 2:
                            dst = qhT[o] if pi == 0 else khT[o]
                            nc.vector.tensor_scalar_add(
                                out=dst[:, sh * NF:(sh + 1) * NF],
                                in0=ps,
                                scalar1=bcol[bnm][:, o:o + 1],
                            )
                        else:
                            vt = vhT_p.tile([P, NF], f32, tag="vt", name="vt")
                            nc.vector.tensor_scalar_add(
                                out=vt, in0=ps, scalar1=bcol[bnm][:, o:o + 1]
                            )
                            # transpose the head-pair half-block into vh layout
                            for sjl in range(NF // P):
                                sj = sh * (NF // P) + sjl
                                tp = tpsum.tile([P, P], f32, tag="tp", name="tp")
                                nc.tensor.transpose(
                                    tp, vt[:, sjl * P:(sjl + 1) * P], ident
                                )
                                for hp in range(2):
                                    nc.vector.tensor_copy(
                                        out=vha[o * 2 + hp][:, sj, 0:DK],
                                        in_=tp[:, hp * DK:(hp + 1) * DK],
                                    )

        # ---------------- phase 2: attention per head ----------------
        ohT_p = ctx.enter_context(tc.tile_pool(name="ohT", bufs=1))
        ohT = [ohT_p.tile([P, S], mdt, name=f"ohT{i}") for i in range(OT)]
        drs_p = ctx.enter_context(tc.tile_pool(name="drs", bufs=2, space="DRAM"))
        with tc.tile_pool(name="stp", bufs=2, space="PSUM") as stp, \
             tc.tile_pool(name="avp", bufs=4, space="PSUM") as avp, \
             tc.tile_pool(name="snp", bufs=1, space="PSUM") as snp, \
             tc.tile_pool(name="atp", bufs=6) as atp, \
             tc.tile_pool(name="anp", bufs=2) as anp, \
             tc.tile_pool(name="anp2", bufs=2) as anp2, \
             tc.tile_pool(name="rbp", bufs=2) as rbp, \
             tc.tile_pool(name="rrp", bufs=2) as rrp, \
             tc.tile_pool(name="smallp", bufs=8) as smallp:

            for h in range(H):
                pair, off = h // 2, (h % 2) * DK
                av = [avp.tile([DK + 1, NF], f32, tag="av", name="av") for _ in range(2)]
                # S1 + S2: transposed scores -> exp -> AV accumulation
                for sj in range(ST):
                    for sih in range(SH):
                        st = stp.tile([P, NF], f32, tag="st", name="st")
                        nc.tensor.matmul(
                            st,
                            lhsT=khT[pair][off:off + DK, sj * P:(sj + 1) * P].bitcast(mdt),
                            rhs=qhT[pair][off:off + DK, sih * NF:(sih + 1) * NF].bitcast(mdt),
                            start=True,
                            stop=True,
                        )
                        at = atp.tile([P, NF], mdt, tag="at", name="at")
                        nc.scalar.activation(out=at, in_=st, func=AF.Exp)
                        nc.tensor.matmul(
                            av[sih],
                            lhsT=vha[h][:, sj, :].bitcast(mdt),
                            rhs=at.bitcast(mdt),
                            start=(sj == 0),
                            stop=(sj == ST - 1),
                        )
                # normalize outhT with broadcast reciprocal of the sums row
                rrow = rrp.tile([1, S], f32, tag="rrow", name="rrow")
                for sih in range(SH):
                    nc.vector.reciprocal(
                        rrow[:, sih * NF:(sih + 1) * NF], av[sih][DK:DK + 1, :]
                    )
                r_sc = drs_p.tile([1, S], f32, tag="rsc", name="rsc")
                nc.sync.dma_start(out=r_sc, in_=rrow)
                rbc = rbp.tile([DK, S], f32, tag="rbc", name="rbc")
                nc.gpsimd.dma_start(out=rbc, in_=r_sc.to_broadcast((DK, S)))
                for sih in range(SH):
                    nc.vector.tensor_mul(
                        out=ohT[pair][off:off + DK, sih * NF:(sih + 1) * NF],
                        in0=av[sih][0:DK, :],
                        in1=rbc[:, sih * NF:(sih + 1) * NF],
                    )
                # S3: normal-orientation scores -> softmax -> attention output
                for si in range(ST):
                    sn = snp.tile([P, S], f32, tag="sn", name="sn")
                    for sjh in range(SH):
                        nc.tensor.matmul(
                            sn[:, sjh * NF:(sjh + 1) * NF],
                            lhsT=qhT[pair][off:off + DK, si * P:(si + 1) * P].bitcast(mdt),
                            rhs=khT[pair][off:off + DK, sjh * NF:(sjh + 1) * NF].bitcast(mdt),
                            start=True,
                            stop=True,
                        )
                    ane = anp.tile([P, S], f32, tag="ane", name="ane")
                    sums = smallp.tile([P, 1], f32, tag="sums", name="sums")
                    nc.scalar.activation(out=ane, in_=sn, func=AF.Exp, accum_out=sums)
                    rec = smallp.tile([P, 1], f32, tag="rec", name="rec")
                    nc.vector.reciprocal(rec, sums)
                    ann = anp2.tile([P, S], f32, tag="ann", name="ann")
                    nc.vector.tensor_scalar_mul(out=ann, in0=ane, scalar1=rec)
                    nc.sync.dma_start(
                        out=att[h, si * P:(si + 1) * P, :], in_=ann
                    )

        # ---------------- phase 3: output projection ----------------
        with tc.tile_pool(name="wop", bufs=OT) as wo_p, \
             tc.tile_pool(name="fpsum", bufs=4, space="PSUM") as fpsum, \
             tc.tile_pool(name="fop", bufs=3) as fo_p:
            wot = [wo_p.tile([P, D], mdt, tag="wot", name="wot") for _ in range(KT)]
            for k_ in range(KT):
                nc.sync.dma_start(out=wot[k_], in_=woT[k_ * P:(k_ + 1) * P, :])
            for s in range(ST):
                for mh in range(SH):
                    fp = fpsum.tile([P, NF], f32, tag="fp", name="fp")
                    for op in range(OT):
                        nc.tensor.matmul(
                            fp,
                            lhsT=ohT[op][:, s * P:(s + 1) * P].bitcast(mdt),
                            rhs=wot[op][:, mh * NF:(mh + 1) * NF].bitcast(mdt),
                            start=(op == 0),
                            stop=(op == OT - 1),
                        )
                    fo = fo_p.tile([P, NF], f32, tag="fo", name="fo")
                    nc.vector.tensor_add(
                        out=fo, in0=fp, in1=bo_bc[:, mh * NF:(mh + 1) * NF]
                    )
                    nc.sync.dma_start(
                        out=out[s * P:(s + 1) * P, mh * NF:(mh + 1) * NF], in_=fo
                    )

    if split_waits:
        _split_multi_waits(nc)
    return nc


_prog_cache = {}


def get_program(mm_dtype_name="float32r"):
    if mm_dtype_name not in _prog_cache:
        _prog_cache[mm_dtype_name] = build_program(mm_dtype_name)
    return _prog_cache[mm_dtype_name]


def make_in_maps(q, k, v, Wq, bq, Wk, bk, Wv, bv, Wo, bo):
    scale = np.float32(1.0 / np.sqrt(DK))
    f = np.float32
    wqT = np.ascontiguousarray(np.asarray(Wq, f).T * scale)
    wkT = np.ascontiguousarray(np.asarray(Wk, f).T)
    wvT = np.ascontiguousarray(np.asarray(Wv, f).T)
    woT = np.ascontiguousarray(np.asarray(Wo, f).T)
    bq_s = np.ascontiguousarray(np.asarray(bq, f) * scale)
    bk_c = np.ascontiguousarray(np.asarray(bk, f))
    bv_c = np.ascontiguousarray(np.asarray(bv, f))
    bo_c = np.ascontiguousarray(np.asarray(bo, f))
    in_maps = []
    for c in range(N_CORES):
        in_maps.append({
            "qT": np.ascontiguousarray(np.asarray(q[c], f).T),
            "kT": np.ascontiguousarray(np.asarray(k[c], f).T),
            "vT": np.ascontiguousarray(np.asarray(v[c], f).T),
            "wqT": wqT, "wkT": wkT, "wvT": wvT, "woT": woT,
            "bq": bq_s, "bk": bk_c, "bv": bv_c, "bo": bo_c,
            "onesd": np.ones((P, ST), np.float32),
        })
    return in_maps


def kernel(q, k, v, Wq, bq, Wk, bk, Wv, bv, Wo, bo):
    from concourse.bass_utils import run_bass_kernel_spmd

    nc = get_program()
    in_maps = make_in_maps(q, k, v, Wq, bq, Wk, bk, Wv, bv, Wo, bo)
    res = run_bass_kernel_spmd(nc, in_maps, list(range(N_CORES)))
    out = np.stack([res.results[c]["out"] for c in range(N_CORES)])
    attention = np.stack([res.results[c]["att"] for c in range(N_CORES)])
    return out, attention
